# revision 3
# baseline (speedup 1.0000x reference)
"""Anisotropic Chebyshev graph convolution on 8 Trainium2 NeuronCores — v2.

  out[b,u,m,n] = sum_{k,l,i,p,q} coefs[k,l,i,u] cheb1[k,p,m] cheb2[l,q,n] x[b,i,p,q]

Sharding: data-parallel over batch B=8, one sample per core; cheb1/cheb2/coefs
replicated (no collectives). Per core, three matmul stages in bf16 (fp32 psum):

  1) a[k,i,m,q]   = sum_p cheb1[k,p,m] x[i,p,q]          (contract p)
  2) c[l,u,m,q]   = sum_{ki} W[ki,lu] a[ki,m,q]          (contract k*C = 160)
  3) out[u,m,n]   = sum_{l,q} c[l,u,m,q] cheb2[l,q,n]    (contract q, acc l)

vs v1: bf16 datapath halves all traffic and lets stage 2 stream W at its true
width (160, no fp32r 256-pad), the `a` bounce is written to DRAM already
transposed ([ki, m, q], so reads are wide/contiguous), DMAs alternate between
the SP and Pool queues, stage-2's ki-tail (rows 128:160) is handled by four
row-packed tile_position matmuls that run concurrently, and the output tensor
is [m, u, n] so each 4-m group stores with one flat DMA (host transposes).
"""

import numpy as np

import concourse.bacc as bacc
import concourse.bass as bass
import concourse.mybir as mybir
import concourse.tile as tile
from concourse import bass_utils

B = 8
C = 32          # input channels i
U = 32          # output units u
K = 5           # chebyshev powers (k and l)
N1 = 256        # first graph axis (p -> m)
N2 = 256        # second graph axis (q -> n)
P = 128

KI = K * C              # 160 mix contraction
LU = K * U              # 160 mix output
N_CORES = 8

F32 = mybir.dt.float32
BF16 = mybir.dt.bfloat16

CHUNK = 512             # stage-1 psum chunk (2 i x 256 q), one f32 bank
NCHUNK = (C * N2) // CHUNK   # 16


def build(n_iters=1):
    nc = bacc.Bacc("TRN2", target_bir_lowering=False, debug=False, num_devices=1)

    x_d = nc.dram_tensor("x", [C, N1, N2], BF16, kind="ExternalInput")
    ch1_d = nc.dram_tensor("cheb1", [K, N1, N1], BF16, kind="ExternalInput")
    ch2_d = nc.dram_tensor("cheb2", [K, N2, N2], BF16, kind="ExternalInput")
    w1_d = nc.dram_tensor("w1", [P, LU], BF16, kind="ExternalInput")
    # w2rep[32j+t, :] = W[128+t, :] for j in 0..4 (tail weights at each row strip)
    w2_d = nc.dram_tensor("w2rep", [P, LU], BF16, kind="ExternalInput")
    out_d = nc.dram_tensor("out", [N1, U, N2], F32, kind="ExternalOutput")

    with tile.TileContext(nc) as tc:
      for _it in range(n_iters):
        with (
            tc.tile_pool(name="const", bufs=1) as const_pool,
            tc.tile_pool(name="adram", bufs=1, space="DRAM") as dram_pool,
            # stage-2/3 SBUF pools live at the outer scope so their addresses
            # don't alias stage-1's ae staging (aliasing would stall the a2a
            # readback until the last aT write releases ae)
            tc.tile_pool(name="a2", bufs=2) as a2_pool,
            tc.tile_pool(name="a2b", bufs=2) as a2b_pool,
            tc.tile_pool(name="csb", bufs=8) as c_pool,
            tc.tile_pool(name="osb", bufs=6) as o_pool,
        ):
            # x as [p, (ph), i*q]
            xp = [
                const_pool.tile([P, C * N2], BF16, tag=f"xp{ph}", name=f"xp{ph}")
                for ph in range(2)
            ]
            x_r = x_d.ap().rearrange("i (ph p) q -> ph p i q", ph=2)
            # cheb1 as [p, k, ph, m] first (stage-1 stationary), then x halves
            # split across three queues so compute starts ~6us in
            ch1 = const_pool.tile([P, K, 2, N1], BF16, tag="ch1")
            nc.sync.dma_start(
                ch1[:], ch1_d.ap().rearrange("k (ph p) m -> p k ph m", ph=2)
            )
            H = (C * N2) // 2
            xv = [xp[ph][:].rearrange("p (i q) -> p i q", q=N2) for ph in range(2)]
            nc.scalar.dma_start(xv[0][:, : C // 4], x_r[0, :, : C // 4])
            nc.sync.dma_start(xv[0][:, C // 4 :], x_r[0, :, C // 4 :])
            nc.gpsimd.dma_start(xv[1][:, : C // 4], x_r[1, :, : C // 4])
            nc.gpsimd.dma_start(xv[1][:, C // 4 :], x_r[1, :, C // 4 :])
            ch2 = const_pool.tile([P, K, 2, N2], BF16, tag="ch2")
            nc.gpsimd.dma_start(
                ch2[:], ch2_d.ap().rearrange("l (qh q) n -> q l qh n", qh=2)
            )
            w1 = const_pool.tile([P, LU], BF16, tag="w1")
            nc.gpsimd.dma_start(w1[:], w1_d.ap())
            w2r = const_pool.tile([P, LU], BF16, tag="w2r")
            nc.gpsimd.dma_start(w2r[:], w2_d.ap())

            # `a` bounce in DRAM, stored TRANSPOSED: [ki, m, q], one tile per
            # m-half (separate tiles so piece readbacks only dep on their half)
            aT = [
                dram_pool.tile([KI, P, N2], BF16, tag=f"aT{mh}", name=f"aT{mh}")
                for mh in range(2)
            ]

            # ---- stage 1: aT[mh, ki, m, q] = cheb1^T x -------------------
            evac_flip = 0
            dma_flip = 0
            with (
                tc.tile_pool(name="ps_a", bufs=4, space="PSUM") as ps_a,
                tc.tile_pool(name="ae", bufs=2) as ae_pool,
            ):
                for mh in range(2):
                    for k in range(K):
                        ae = ae_pool.tile([P, C, N2], BF16, tag="ae")
                        for cg in range(NCHUNK // 2):
                            ps = ps_a.tile([P, 2 * CHUNK], F32, tag="ps_a")
                            for ph in range(2):
                                lhsT = ch1[:, k, ph, mh * P : (mh + 1) * P]
                                for cj in range(2):
                                    ci = cg * 2 + cj
                                    nc.tensor.matmul(
                                        ps[:, cj * CHUNK : (cj + 1) * CHUNK],
                                        lhsT,
                                        xp[ph][:, ci * CHUNK : (ci + 1) * CHUNK],
                                        start=(ph == 0),
                                        stop=(ph == 1),
                                    )
                            dst = ae[:, 4 * cg : 4 * cg + 4, :]
                            if evac_flip == 0:
                                nc.vector.tensor_copy(dst, ps[:])
                            else:
                                nc.scalar.copy(dst, ps[:])
                            evac_flip ^= 1
                        # write transposed: aT[mh, k*C:(k+1)*C, :, :] <- ae.
                        # mh0 writes on sync, mh1 on gpsimd, so each piece's
                        # readback (below) FIFOs behind only the writes it
                        # actually depends on.
                        dst_ap = aT[mh][k * C : (k + 1) * C, :, :].rearrange(
                            "i m q -> m i q"
                        )
                        eng = nc.sync if mh == 0 else nc.gpsimd
                        eng.dma_start(dst_ap, ae[:])

            # ---- stages 2+3 per 64-m piece -------------------------------
            with (
                tc.tile_pool(name="ps_c", bufs=6, space="PSUM") as ps_c,
                tc.tile_pool(name="ps_o", bufs=2, space="PSUM") as ps_o,
            ):
                MB = 64                   # m's per piece
                GPP = MB // 4             # 4-m groups per piece
                for piece in range(N1 // MB):
                    mh, m0 = piece // 2, (piece % 2) * MB
                    # readback queue matches the writer queue of this mh
                    reng = nc.sync if mh == 0 else nc.gpsimd
                    a2a = a2_pool.tile([P, MB, N2], BF16, tag="a2a")
                    reng.dma_start(a2a[:], aT[mh][:P, m0 : m0 + MB, :])
                    # tail: partition 32j+t holds ki=128+t for m%4==j
                    a2b = a2b_pool.tile([P, GPP, N2], BF16, tag="a2b")
                    for j in range(4):
                        src = aT[mh][P:KI, m0 + j : m0 + MB : 4, :]
                        reng.dma_start(a2b[32 * j : 32 * (j + 1), :, :], src)
                    for g in range(GPP):
                        c_tiles = []
                        for qh in range(2):
                            csb = c_pool.tile([P, K, 4, U], BF16, tag="csb")
                            c_tiles.append(csb)
                            cps_l = []
                            for mj in range(4):
                                cps = ps_c.tile([P, LU], F32, tag="ps_c")
                                cps_l.append(cps)
                                nc.tensor.matmul(
                                    cps[:],
                                    a2a[:, g * 4 + mj, qh * P : (qh + 1) * P],
                                    w1[:],
                                    start=True,
                                    stop=False,
                                )
                            for mj in range(4):
                                nc.tensor.matmul(
                                    cps_l[mj][:],
                                    a2b[32 * mj : 32 * mj + 32, g, qh * P : (qh + 1) * P],
                                    w2r[32 * mj : 32 * mj + 32, :],
                                    start=False,
                                    stop=True,
                                    tile_position=(32 * mj, 0),
                                )
                            for mj in range(4):
                                src = cps_l[mj][:].rearrange("p (l u) -> p l u", u=U)
                                dst = csb[:, :, mj, :]
                                if evac_flip == 0:
                                    nc.vector.tensor_copy(dst, src)
                                else:
                                    nc.scalar.copy(dst, src)
                                evac_flip ^= 1

                        # stage 3: out[(mj,u), n] += c^T cheb2
                        ops = ps_o.tile([P, N2], F32, tag="ps_o")
                        for l in range(K):
                            for qh in range(2):
                                nc.tensor.matmul(
                                    ops[:],
                                    c_tiles[qh][:, l],
                                    ch2[:, l, qh, :],
                                    start=(l == 0 and qh == 0),
                                    stop=(l == K - 1 and qh == 1),
                                )
                        osb = o_pool.tile([P, N2], F32, tag="osb")
                        if evac_flip == 0:
                            nc.vector.tensor_copy(osb[:], ops[:])
                        else:
                            nc.scalar.copy(osb[:], ops[:])
                        evac_flip ^= 1
                        m_abs = piece * MB + g * 4
                        dst = out_d.ap()[m_abs : m_abs + 4, :, :].rearrange(
                            "m u n -> (m u) n"
                        )
                        # opposite queue of this mh's reads, so stores don't
                        # delay the next piece's readback
                        oeng = nc.gpsimd if mh == 0 else nc.sync
                        oeng.dma_start(dst, osb[:])

    nc.compile()
    return nc


_NC = None
LAST_RUN = {}


def _bf16(a):
    import ml_dtypes

    return np.asarray(a, dtype=np.float32).astype(ml_dtypes.bfloat16)


def _weights(coefs):
    w = np.asarray(coefs, np.float32).transpose(0, 2, 1, 3).reshape(KI, LU)
    w1 = w[:P]
    w2rep = np.tile(w[P:KI], (4, 1))
    return _bf16(np.ascontiguousarray(w1)), _bf16(np.ascontiguousarray(w2rep))


def core_input_map(x, cheb1, cheb2, coefs, core):
    w1, w2rep = _weights(coefs)
    return {
        "x": _bf16(np.asarray(x, np.float32)[core]),
        "cheb1": _bf16(cheb1),
        "cheb2": _bf16(cheb2),
        "w1": w1,
        "w2rep": w2rep,
    }


def core_expected(expected, core):
    return np.asarray(expected)[core].transpose(1, 0, 2)  # [u,m,n] -> [m,u,n]


def kernel(x, cheb1, cheb2, coefs):
    global _NC
    import time as _time

    if _NC is None:
        t0 = _time.monotonic()
        _NC = build()
        LAST_RUN["build_s"] = _time.monotonic() - t0

    w1, w2rep = _weights(coefs)
    ch1b, ch2b = _bf16(cheb1), _bf16(cheb2)
    xb = _bf16(x)

    in_maps = [
        {"x": xb[b], "cheb1": ch1b, "cheb2": ch2b, "w1": w1, "w2rep": w2rep}
        for b in range(B)
    ]

    t0 = _time.monotonic()
    res = bass_utils.run_bass_kernel_spmd(_NC, in_maps, core_ids=list(range(N_CORES)))
    LAST_RUN["wall_s"] = _time.monotonic() - t0
    LAST_RUN["exec_time_ns"] = res.exec_time_ns

    # out is [m, u, n] per core -> [u, m, n]
    return np.stack(
        [res.results[b]["out"].transpose(1, 0, 2) for b in range(B)]
    )
